# revision 9
# baseline (speedup 1.0000x reference)
"""Trainium2 Bass kernel for nn_DifferentiableRollout.

Computes, for B=1024 batched rollouts of T=200 steps:
    x_{t+1} = x_t + DT * ( tanh(concat(x_t, u_t) @ W1 + b1) @ W2 + b2 )
returning all states [B, T+1, SD].

Strategy (8 NeuronCores, data-parallel over batch, 128 rows/core):
  * The rollout is reformulated so the per-step critical chain has only
    TWO cross-engine hops.  With pre_t = xu_t @ W1 + b1 (the
    preactivation) and W12 = DT * (W2 @ W1x):
        h_t       = tanh(pre_t)                               (ACT)
        pre_{t+1} = pre_t + h_t @ W12 + du_{t+1} @ W1u + c    (PE, 20 mm)
    where du_{t+1} = u_{t+1} - u_t is host-precomputed and c = DT*b2@W1x.
    pre lives in PSUM as a never-restarting accumulation group, so the
    chain is ACT -> PE -> ACT; the naive path's mm2 -> DVE(x slot) -> mm1
    round trip (two extra cross-engine hops with their access-latency
    tails) is gone.
  * The states themselves are NOT computed on device.  h_t is already in
    SBUF (a 16-slot ring per stream), and chunked DMAs stream all h out;
    the host finishes with x = x0 + DT*cumsum_t(h @ W2 + b2) in fp32 --
    exact given the h sequence, and off the device's critical path.
    This removes the per-step W2 matmuls and the DVE slot-write: the PE
    burst is only the 20 recurrence matmuls.
  * Per core, the 128 batch rows split into STREAMS independent streams
    whose chains interleave so engine time hides inside the chain
    latency of the other streams.
  * PSUM start=True marks a whole 2KB zero region, so each psum tile is
    zeroed by ONE start=True matmul (zero weights); everything after
    accumulates with start=False.
  * fp16 matmul inputs + fp32 PSUM accumulation: rel err ~1.9e-3.
"""

from contextlib import ExitStack

import numpy as np

import concourse.bacc as bacc
import concourse.bass as bass
import concourse.mybir as mybir
import concourse.tile as tile
from concourse.bass_utils import run_bass_kernel_spmd

B, T, SD, CD, H = 1024, 200, 64, 32, 512
DT = 0.1
NCORES = 8
STREAMS = 3
BLOCAL = B // NCORES          # 128 batch rows per core
WIDTHS = [BLOCAL // STREAMS + (1 if s < BLOCAL % STREAMS else 0) for s in range(STREAMS)]
OFFS = [sum(WIDTHS[:s]) for s in range(STREAMS)]
HCH = H // 128                # 4 feature chunks
KX = SD + 1 + CD              # 97 rows of the seed xu vector
KD = CD + 1                   # 33 rows of the du plane (du + ones)
RING = 16                     # h slots per stream between out-DMAs

F16 = mybir.dt.float16
F32 = mybir.dt.float32


def _build_module(t_steps: int = T, streams: int = STREAMS, widths=None):
    """Build + compile the per-core Bass module (SPMD: same NEFF, 8 cores)."""
    if widths is None:
        widths = WIDTHS
    ts = bass.ts
    nc = bacc.Bacc(
        "TRN2",
        target_bir_lowering=False,
        debug=False,
        enable_asserts=False,
        num_devices=NCORES,
    )

    d_w12 = nc.dram_tensor("w12", [128, HCH * HCH * 128], F16, kind="ExternalInput")
    d_w1du = nc.dram_tensor("w1du", [KD, HCH * 128], F16, kind="ExternalInput")
    d_w1 = nc.dram_tensor("w1aug", [128, H], F16, kind="ExternalInput")
    d_xu0, d_du, d_h = [], [], []
    for s in range(streams):
        w = widths[s]
        d_xu0.append(nc.dram_tensor(f"xu0_{s}", [128, w], F16, kind="ExternalInput"))
        d_du.append(
            nc.dram_tensor(f"du{s}", [KD, (t_steps - 1) * w], F16, kind="ExternalInput")
        )
        d_h.append(
            nc.dram_tensor(f"h{s}", [128, t_steps * HCH * w], F16, kind="ExternalOutput")
        )

    with tile.TileContext(nc) as tc, ExitStack() as ctx:
        const = ctx.enter_context(tc.tile_pool(name="const", bufs=1))
        psum = ctx.enter_context(tc.tile_pool(name="psum", bufs=1, space="PSUM"))

        w12_sb = const.tile([128, HCH * HCH * 128], F16)
        w1du_sb = const.tile([KD, HCH * 128], F16)
        w1_sb = const.tile([128, H], F16)
        xu0_sb, du_sb, h_sb = [], [], []
        for s in range(streams):
            w = widths[s]
            xu0_sb.append(const.tile([128, w], F16, name=f"xu0_{s}"))
            du_sb.append(const.tile([KD, (t_steps - 1) * w], F16, name=f"du{s}"))
            h_sb.append(const.tile([128, RING * HCH * w], F16, name=f"h{s}"))

        psum_pre = []
        for s in range(streams):
            w = widths[s]
            psum_pre.append(psum.tile([128, HCH * w], F32, tag=f"pre{s}", name=f"pre{s}"))

        # Startup DMAs in dependency-deadline order: seeds first, then the
        # small first du chunk (needed at burst 0), then the weights the
        # first step consumes, then the rest of du round-robin.
        for s in range(streams):
            nc.sync.dma_start(xu0_sb[s][:], d_xu0[s].ap()[:])
        first = min(8, t_steps - 1)
        for s in range(streams):
            w = widths[s]
            nc.gpsimd.dma_start(du_sb[s][:, 0 : first * w], d_du[s].ap()[:, 0 : first * w])
        nc.gpsimd.dma_start(w1_sb[:], d_w1.ap()[:])
        nc.sync.dma_start(w12_sb[:], d_w12.ap()[:])
        nc.gpsimd.dma_start(w1du_sb[:], d_w1du.ap()[:])
        cb = np.linspace(first, t_steps - 1, 5).astype(int)
        for k in range(len(cb) - 1):
            a, b = int(cb[k]), int(cb[k + 1])
            if a == b:
                continue
            for s in range(streams):
                w = widths[s]
                nc.sync.dma_start(
                    du_sb[s][:, a * w : b * w], d_du[s].ap()[:, a * w : b * w]
                )

        # Warm-up: pull the ACT tanh table load off step 0's critical path
        # and give the PE p-state ramp something to chew on.
        warm_ps = psum.tile([SD, SD], F32, tag="warm", name="warm_ps")
        warm_sb = const.tile([SD, SD], F16, name="warm_sb")
        zero_sb = const.tile([128, 256], F16, name="zero_sb")
        nc.gpsimd.memset(zero_sb[:], 0.0)
        for _ in range(4):
            nc.tensor.matmul(
                warm_ps[:], zero_sb[:, 0:SD], zero_sb[:, 0:SD], start=True, stop=True
            )
        nc.scalar.activation(warm_sb[:], warm_ps[:], mybir.ActivationFunctionType.Tanh)

        # Zero psum_pre with a single start=True matmul per tile, then seed
        # pre_0 = W1aug.T @ [x0; 1; u0] with accumulating matmuls.
        for s in range(streams):
            w = widths[s]
            nc.tensor.matmul(
                psum_pre[s][:], zero_sb[:, 0:128], zero_sb[:, 0 : HCH * w],
                start=True, stop=False, skip_group_check=True,
            )
        for s in range(streams):
            w = widths[s]
            for j in range(HCH):
                nc.tensor.matmul(
                    psum_pre[s][:, ts(j, w)],
                    w1_sb[:, ts(j, 128)],
                    xu0_sb[s][:],
                    start=False, stop=False, skip_group_check=True,
                )

        for t in range(t_steps):
            last_t = t == t_steps - 1
            r = t % RING
            for s in range(streams):
                w = widths[s]
                hs = h_sb[s][:, (r * HCH) * w : (r * HCH + HCH) * w]
                nc.scalar.activation(
                    hs, psum_pre[s][:], mybir.ActivationFunctionType.Tanh
                )
                if not last_t:
                    # recurrence burst: pre += h @ W12 (16 mm) then the
                    # du/bias inject (4 mm); the next tanh waits on these.
                    for j in range(HCH):
                        for i in range(HCH):
                            nc.tensor.matmul(
                                psum_pre[s][:, ts(j, w)],
                                w12_sb[:, ts(j * HCH + i, 128)],
                                h_sb[s][:, (r * HCH + i) * w : (r * HCH + i + 1) * w],
                                start=False, stop=False, skip_group_check=True,
                            )
                    for j in range(HCH):
                        nc.tensor.matmul(
                            psum_pre[s][:, ts(j, w)],
                            w1du_sb[:, ts(j, 128)],
                            du_sb[s][:, ts(t, w)],
                            start=False, stop=t == t_steps - 2,
                            skip_group_check=True,
                        )
            # drain full rings (and the final partial ring) to DRAM
            if r == RING - 1 or last_t:
                lo_t = t - r
                for s in range(streams):
                    w = widths[s]
                    nc.sync.dma_start(
                        d_h[s].ap()[:, lo_t * HCH * w : (t + 1) * HCH * w],
                        h_sb[s][:, 0 : (r + 1) * HCH * w],
                    )

    nc.compile()
    return nc


_CACHE: dict = {}


def _get_module():
    if "nc" not in _CACHE:
        _CACHE["nc"] = _build_module()
    return _CACHE["nc"]


def _prep_inputs(x0, controls, W1, b1, W2, b2):
    """Host-side prep: shard, transpose, augment, cast. Returns in_maps."""
    f16 = np.float16
    W1 = np.asarray(W1, np.float32)
    b1 = np.asarray(b1, np.float32)
    W2 = np.asarray(W2, np.float32)
    b2 = np.asarray(b2, np.float32)
    x0 = np.asarray(x0, np.float32)
    controls = np.asarray(controls, np.float32)
    W1x, W1u = W1[:SD], W1[SD:]

    # W12 = DT * W2 @ W1x, chunked for the PE: w12[p, j, i, c] = W12[128i+p, 128j+c]
    W12 = (DT * (W2 @ W1x)).astype(np.float32)
    w12 = W12.reshape(HCH, 128, HCH, 128).transpose(1, 2, 0, 3)
    w12 = np.ascontiguousarray(w12).reshape(128, HCH * HCH * 128).astype(f16)
    # du-inject weights: rows 0..31 = W1u, row 32 = DT * b2 @ W1x
    cvec = (DT * (b2 @ W1x)).astype(np.float32)
    w1du = np.concatenate([W1u, cvec[None, :]], axis=0).astype(f16)  # [33, 512]
    w1aug = np.concatenate(
        [W1x, b1[None, :], W1u, np.zeros((128 - KX, H), np.float32)], axis=0
    ).astype(f16)

    x0T = x0.T.astype(f16)                              # [SD, B]
    u0T = controls[:, 0, :].T.astype(f16)               # [CD, B]
    du = controls[:, 1:, :] - controls[:, :-1, :]       # [B, T-1, CD] f32
    duT = du.transpose(2, 1, 0).astype(f16)             # [CD, T-1, B]

    in_maps = []
    for c in range(NCORES):
        m = {"w12": w12, "w1du": w1du, "w1aug": w1aug}
        for s in range(STREAMS):
            w = WIDTHS[s]
            lo = c * BLOCAL + OFFS[s]
            cols = slice(lo, lo + w)
            xu0 = np.zeros((128, w), f16)
            xu0[0:SD] = x0T[:, cols]
            xu0[SD] = 1.0
            xu0[SD + 1 : KX] = u0T[:, cols]
            m[f"xu0_{s}"] = xu0
            dstream = np.concatenate(
                [duT[:, :, cols], np.ones((1, T - 1, w), f16)], axis=0
            )  # [33, T-1, w]
            m[f"du{s}"] = np.ascontiguousarray(dstream).reshape(KD, (T - 1) * w)
        in_maps.append(m)
    return in_maps


def kernel(x0, controls, W1, b1, W2, b2):
    nc = _get_module()
    in_maps = _prep_inputs(x0, controls, W1, b1, W2, b2)
    res = run_bass_kernel_spmd(nc, in_maps, core_ids=list(range(NCORES)))

    x0 = np.asarray(x0, np.float32)
    W2f = np.asarray(W2, np.float32)
    b2f = np.asarray(b2, np.float32)
    states = np.empty((B, T + 1, SD), np.float32)
    states[:, 0, :] = x0
    for c in range(NCORES):
        for s in range(STREAMS):
            w = WIDTHS[s]
            lo = c * BLOCAL + OFFS[s]
            hraw = np.asarray(res.results[c][f"h{s}"], np.float16)
            # h[p, t, i, col] -> H[t, col, 128i+p]
            Hs = hraw.reshape(128, T, HCH, w).transpose(1, 3, 2, 0).reshape(T, w, H)
            v = Hs.astype(np.float32) @ (DT * W2f) + DT * b2f  # [T, w, SD]
            states[lo : lo + w, 1:] = x0[lo : lo + w, None, :] + np.cumsum(
                v, axis=0
            ).transpose(1, 0, 2)
    return states


# revision 11
# speedup vs baseline: 1.4123x; 1.4123x over previous
"""Trainium2 Bass kernel for nn_DifferentiableRollout.

Computes, for B=1024 batched rollouts of T=200 steps:
    x_{t+1} = x_t + DT * ( tanh(concat(x_t, u_t) @ W1 + b1) @ W2 + b2 )
returning all states [B, T+1, SD].

Strategy (8 NeuronCores, data-parallel over batch, 128 rows/core):
  * The rollout is reformulated so the per-step critical chain has only
    TWO cross-engine hops.  With pre_t = xu_t @ W1 + b1 (the
    preactivation) and W12 = DT * (W2 @ W1x):
        h_t       = tanh(pre_t)                               (ACT)
        pre_{t+1} = pre_t + h_t @ W12 + du_{t+1} @ W1u + c    (PE, 20 mm)
    where du_{t+1} = u_{t+1} - u_t is host-precomputed and c = DT*b2@W1x.
    pre lives in PSUM as a never-restarting accumulation group, so the
    chain is ACT -> PE -> ACT; the naive path's mm2 -> DVE(x slot) -> mm1
    round trip (two extra cross-engine hops with their access-latency
    tails) is gone.
  * The states themselves are NOT computed on device.  h_t is already in
    SBUF (a 16-slot ring per stream), and chunked DMAs stream all h out;
    the host finishes with x = x0 + DT*cumsum_t(h @ W2 + b2) in fp32 --
    exact given the h sequence, and off the device's critical path.
    This removes the per-step W2 matmuls and the DVE slot-write: the PE
    burst is only the 20 recurrence matmuls.
  * Per core, the 128 batch rows split into STREAMS independent streams
    whose chains interleave so engine time hides inside the chain
    latency of the other streams.
  * PSUM start=True marks a whole 2KB zero region, so each psum tile is
    zeroed by ONE start=True matmul (zero weights); everything after
    accumulates with start=False.
  * fp16 matmul inputs + fp32 PSUM accumulation: rel err ~1.9e-3.
"""

from contextlib import ExitStack

import numpy as np

import concourse.bacc as bacc
import concourse.bass as bass
import concourse.mybir as mybir
import concourse.tile as tile
from concourse.bass_utils import run_bass_kernel_spmd

B, T, SD, CD, H = 1024, 200, 64, 32, 512
DT = 0.1
NCORES = 8
STREAMS = 3
BLOCAL = B // NCORES          # 128 batch rows per core
WIDTHS = [BLOCAL // STREAMS + (1 if s < BLOCAL % STREAMS else 0) for s in range(STREAMS)]
OFFS = [sum(WIDTHS[:s]) for s in range(STREAMS)]
HCH = H // 128                # 4 feature chunks
KX = SD + 1 + CD              # 97 rows of the seed xu vector
KD = CD + 1                   # 33 rows of the du plane (du + ones)
RING = 16                     # h slots per stream between out-DMAs

F16 = mybir.dt.float16
F32 = mybir.dt.float32


def _build_module(t_steps: int = T, streams: int = STREAMS, widths=None):
    """Build + compile the per-core Bass module (SPMD: same NEFF, 8 cores)."""
    if widths is None:
        widths = WIDTHS
    ts = bass.ts
    nc = bacc.Bacc(
        "TRN2",
        target_bir_lowering=False,
        debug=False,
        enable_asserts=False,
        num_devices=NCORES,
    )

    d_w12 = nc.dram_tensor("w12", [128, HCH * HCH * 128], F16, kind="ExternalInput")
    d_w1du = nc.dram_tensor("w1du", [KD, HCH * 128], F16, kind="ExternalInput")
    d_w1 = nc.dram_tensor("w1aug", [128, H], F16, kind="ExternalInput")
    d_xu0, d_du, d_h = [], [], []
    for s in range(streams):
        w = widths[s]
        d_xu0.append(nc.dram_tensor(f"xu0_{s}", [128, w], F16, kind="ExternalInput"))
        d_du.append(
            nc.dram_tensor(f"du{s}", [KD, (t_steps - 1) * w], F16, kind="ExternalInput")
        )
        d_h.append(
            nc.dram_tensor(f"h{s}", [128, t_steps * HCH * w], F16, kind="ExternalOutput")
        )

    with tile.TileContext(nc) as tc, ExitStack() as ctx:
        const = ctx.enter_context(tc.tile_pool(name="const", bufs=1))
        psum = ctx.enter_context(tc.tile_pool(name="psum", bufs=1, space="PSUM"))

        w12_sb = const.tile([128, HCH * HCH * 128], F16)
        w1du_sb = const.tile([KD, HCH * 128], F16)
        w1_sb = const.tile([128, H], F16)
        xu0_sb, du_sb, h_sb = [], [], []
        for s in range(streams):
            w = widths[s]
            xu0_sb.append(const.tile([128, w], F16, name=f"xu0_{s}"))
            du_sb.append(const.tile([KD, (t_steps - 1) * w], F16, name=f"du{s}"))
            h_sb.append(const.tile([128, RING * HCH * w], F16, name=f"h{s}"))

        psum_pre = []
        for s in range(streams):
            w = widths[s]
            psum_pre.append(psum.tile([128, HCH * w], F32, tag=f"pre{s}", name=f"pre{s}"))

        # Startup DMAs in dependency-deadline order: seeds first, then the
        # small first du chunk (needed at burst 0), then the weights the
        # first step consumes, then the rest of du round-robin.
        for s in range(streams):
            nc.sync.dma_start(xu0_sb[s][:], d_xu0[s].ap()[:])
        first = min(8, t_steps - 1)
        for s in range(streams):
            w = widths[s]
            nc.gpsimd.dma_start(du_sb[s][:, 0 : first * w], d_du[s].ap()[:, 0 : first * w])
        nc.gpsimd.dma_start(w1_sb[:], d_w1.ap()[:])
        nc.sync.dma_start(w12_sb[:], d_w12.ap()[:])
        nc.gpsimd.dma_start(w1du_sb[:], d_w1du.ap()[:])
        cb = np.linspace(first, t_steps - 1, 5).astype(int)
        for k in range(len(cb) - 1):
            a, b = int(cb[k]), int(cb[k + 1])
            if a == b:
                continue
            for s in range(streams):
                w = widths[s]
                nc.sync.dma_start(
                    du_sb[s][:, a * w : b * w], d_du[s].ap()[:, a * w : b * w]
                )

        # Warm-up: pull the ACT tanh table load off step 0's critical path
        # and give the PE p-state ramp something to chew on.
        warm_ps = psum.tile([SD, SD], F32, tag="warm", name="warm_ps")
        warm_sb = const.tile([SD, SD], F16, name="warm_sb")
        zero_sb = const.tile([128, 256], F16, name="zero_sb")
        nc.gpsimd.memset(zero_sb[:], 0.0)
        for _ in range(4):
            nc.tensor.matmul(
                warm_ps[:], zero_sb[:, 0:SD], zero_sb[:, 0:SD], start=True, stop=True
            )
        nc.scalar.activation(warm_sb[:], warm_ps[:], mybir.ActivationFunctionType.Tanh)

        # Zero psum_pre with a single start=True matmul per tile, then seed
        # pre_0 = W1aug.T @ [x0; 1; u0] with accumulating matmuls.
        for s in range(streams):
            w = widths[s]
            nc.tensor.matmul(
                psum_pre[s][:], zero_sb[:, 0:128], zero_sb[:, 0 : HCH * w],
                start=True, stop=False, skip_group_check=True,
            )
        for s in range(streams):
            w = widths[s]
            for j in range(HCH):
                nc.tensor.matmul(
                    psum_pre[s][:, ts(j, w)],
                    w1_sb[:, ts(j, 128)],
                    xu0_sb[s][:],
                    start=False, stop=False, skip_group_check=True,
                )

        for t in range(t_steps):
            last_t = t == t_steps - 1
            r = t % RING
            for s in range(streams):
                w = widths[s]
                hs = h_sb[s][:, (r * HCH) * w : (r * HCH + HCH) * w]
                nc.scalar.activation(
                    hs, psum_pre[s][:], mybir.ActivationFunctionType.Tanh
                )
                if not last_t:
                    # recurrence burst: pre += h @ W12 (16 mm) then the
                    # du/bias inject (4 mm); the next tanh waits on these.
                    for j in range(HCH):
                        for i in range(HCH):
                            nc.tensor.matmul(
                                psum_pre[s][:, ts(j, w)],
                                w12_sb[:, ts(j * HCH + i, 128)],
                                h_sb[s][:, (r * HCH + i) * w : (r * HCH + i + 1) * w],
                                start=False, stop=False, skip_group_check=True,
                            )
                    for j in range(HCH):
                        nc.tensor.matmul(
                            psum_pre[s][:, ts(j, w)],
                            w1du_sb[:, ts(j, 128)],
                            du_sb[s][:, ts(t, w)],
                            start=False, stop=t == t_steps - 2,
                            skip_group_check=True,
                        )
            # Drain the ring in aligned half-chunks, one DMA queue per
            # stream (SP/DVE/Pool).  A half drained at slot 7 is rewritten
            # 9 rounds later, so the WAR on the ring never stalls a tanh.
            if r % (RING // 2) == RING // 2 - 1 or last_t:
                lo_r = r - (r % (RING // 2))
                lo_t = t - (r % (RING // 2))
                for s in range(streams):
                    w = widths[s]
                    q = (nc.sync, nc.gpsimd, nc.sync)[s % 3]
                    q.dma_start(
                        d_h[s].ap()[:, lo_t * HCH * w : (t + 1) * HCH * w],
                        h_sb[s][:, lo_r * HCH * w : (r + 1) * HCH * w],
                    )

    nc.compile()
    return nc


_CACHE: dict = {}


def _get_module():
    if "nc" not in _CACHE:
        _CACHE["nc"] = _build_module()
    return _CACHE["nc"]


def _prep_inputs(x0, controls, W1, b1, W2, b2):
    """Host-side prep: shard, transpose, augment, cast. Returns in_maps."""
    f16 = np.float16
    W1 = np.asarray(W1, np.float32)
    b1 = np.asarray(b1, np.float32)
    W2 = np.asarray(W2, np.float32)
    b2 = np.asarray(b2, np.float32)
    x0 = np.asarray(x0, np.float32)
    controls = np.asarray(controls, np.float32)
    W1x, W1u = W1[:SD], W1[SD:]

    # W12 = DT * W2 @ W1x, chunked for the PE: w12[p, j, i, c] = W12[128i+p, 128j+c]
    W12 = (DT * (W2 @ W1x)).astype(np.float32)
    w12 = W12.reshape(HCH, 128, HCH, 128).transpose(1, 2, 0, 3)
    w12 = np.ascontiguousarray(w12).reshape(128, HCH * HCH * 128).astype(f16)
    # du-inject weights: rows 0..31 = W1u, row 32 = DT * b2 @ W1x
    cvec = (DT * (b2 @ W1x)).astype(np.float32)
    w1du = np.concatenate([W1u, cvec[None, :]], axis=0).astype(f16)  # [33, 512]
    w1aug = np.concatenate(
        [W1x, b1[None, :], W1u, np.zeros((128 - KX, H), np.float32)], axis=0
    ).astype(f16)

    x0T = x0.T.astype(f16)                              # [SD, B]
    u0T = controls[:, 0, :].T.astype(f16)               # [CD, B]
    du = controls[:, 1:, :] - controls[:, :-1, :]       # [B, T-1, CD] f32
    duT = du.transpose(2, 1, 0).astype(f16)             # [CD, T-1, B]

    in_maps = []
    for c in range(NCORES):
        m = {"w12": w12, "w1du": w1du, "w1aug": w1aug}
        for s in range(STREAMS):
            w = WIDTHS[s]
            lo = c * BLOCAL + OFFS[s]
            cols = slice(lo, lo + w)
            xu0 = np.zeros((128, w), f16)
            xu0[0:SD] = x0T[:, cols]
            xu0[SD] = 1.0
            xu0[SD + 1 : KX] = u0T[:, cols]
            m[f"xu0_{s}"] = xu0
            dstream = np.concatenate(
                [duT[:, :, cols], np.ones((1, T - 1, w), f16)], axis=0
            )  # [33, T-1, w]
            m[f"du{s}"] = np.ascontiguousarray(dstream).reshape(KD, (T - 1) * w)
        in_maps.append(m)
    return in_maps


def kernel(x0, controls, W1, b1, W2, b2):
    nc = _get_module()
    in_maps = _prep_inputs(x0, controls, W1, b1, W2, b2)
    res = run_bass_kernel_spmd(nc, in_maps, core_ids=list(range(NCORES)))

    x0 = np.asarray(x0, np.float32)
    W2f = np.asarray(W2, np.float32)
    b2f = np.asarray(b2, np.float32)
    states = np.empty((B, T + 1, SD), np.float32)
    states[:, 0, :] = x0
    for c in range(NCORES):
        for s in range(STREAMS):
            w = WIDTHS[s]
            lo = c * BLOCAL + OFFS[s]
            hraw = np.asarray(res.results[c][f"h{s}"], np.float16)
            # h[p, t, i, col] -> H[t, col, 128i+p]
            Hs = hraw.reshape(128, T, HCH, w).transpose(1, 3, 2, 0).reshape(T, w, H)
            v = Hs.astype(np.float32) @ (DT * W2f) + DT * b2f  # [T, w, SD]
            states[lo : lo + w, 1:] = x0[lo : lo + w, None, :] + np.cumsum(
                v, axis=0
            ).transpose(1, 0, 2)
    return states


# revision 19
# speedup vs baseline: 1.4413x; 1.0206x over previous
"""Trainium2 Bass kernel for nn_DifferentiableRollout.

Computes, for B=1024 batched rollouts of T=200 steps:
    x_{t+1} = x_t + DT * ( tanh(concat(x_t, u_t) @ W1 + b1) @ W2 + b2 )
returning all states [B, T+1, SD].

Strategy (8 NeuronCores, data-parallel over batch, 128 rows/core):
  * The rollout is reformulated so the per-step critical chain has only
    TWO cross-engine hops.  With pre_t = xu_t @ W1 + b1 (the
    preactivation) and W12 = DT * (W2 @ W1x):
        h_t       = tanh(pre_t)                               (ACT)
        pre_{t+1} = pre_t + h_t @ W12 + du_{t+1} @ W1u + c    (PE, 20 mm)
    where du_{t+1} = u_{t+1} - u_t is host-precomputed and c = DT*b2@W1x.
    pre lives in PSUM as a never-restarting accumulation group, so the
    chain is ACT -> PE -> ACT; the naive path's mm2 -> DVE(x slot) -> mm1
    round trip (two extra cross-engine hops with their access-latency
    tails) is gone.
  * The states themselves are NOT computed on device.  h_t is already in
    SBUF (a 16-slot ring per stream), and chunked DMAs stream all h out;
    the host finishes with x = x0 + DT*cumsum_t(h @ W2 + b2) in fp32 --
    exact given the h sequence, and off the device's critical path.
    This removes the per-step W2 matmuls and the DVE slot-write: the PE
    burst is only the 20 recurrence matmuls.
  * Per core, the 128 batch rows split into STREAMS independent streams
    whose chains interleave so engine time hides inside the chain
    latency of the other streams.
  * PSUM start=True marks a whole 2KB zero region, so each psum tile is
    zeroed by ONE start=True matmul (zero weights); everything after
    accumulates with start=False.
  * fp16 matmul inputs + fp32 PSUM accumulation: rel err ~1.9e-3.
"""

from contextlib import ExitStack

import numpy as np

import concourse.bacc as bacc
import concourse.bass as bass
import concourse.mybir as mybir
import concourse.tile as tile
from concourse.bass_utils import run_bass_kernel_spmd

B, T, SD, CD, H = 1024, 200, 64, 32, 512
DT = 0.1
NCORES = 8
STREAMS = 3
BLOCAL = B // NCORES          # 128 batch rows per core
WIDTHS = [BLOCAL // STREAMS + (1 if s < BLOCAL % STREAMS else 0) for s in range(STREAMS)]
OFFS = [sum(WIDTHS[:s]) for s in range(STREAMS)]
HCH = H // 128                # 4 feature chunks
KX = SD + 1 + CD              # 97 rows of the seed xu vector
KD = CD + 1                   # 33 rows of the du plane (du + ones)
RING = 32                     # h ring slots per stream (drained in halves)

F16 = mybir.dt.float16
F32 = mybir.dt.float32


def _build_module(t_steps: int = T, streams: int = STREAMS, widths=None):
    """Build + compile the per-core Bass module (SPMD: same NEFF, 8 cores)."""
    if widths is None:
        widths = WIDTHS
    ts = bass.ts
    nc = bacc.Bacc(
        "TRN2",
        target_bir_lowering=False,
        debug=False,
        enable_asserts=False,
        num_devices=NCORES,
    )

    d_w12 = nc.dram_tensor("w12", [128, HCH * HCH * 128], F16, kind="ExternalInput")
    d_w1du = nc.dram_tensor("w1du", [KD, HCH * 128], F16, kind="ExternalInput")
    d_w1 = nc.dram_tensor("w1aug", [128, H], F16, kind="ExternalInput")
    d_xu0 = nc.dram_tensor("xu0", [128, BLOCAL], F16, kind="ExternalInput")
    d_du = nc.dram_tensor("du", [KD, (t_steps - 1) * BLOCAL], F16, kind="ExternalInput")
    d_h = []
    for s in range(streams):
        w = widths[s]
        d_h.append(
            nc.dram_tensor(f"h{s}", [128, t_steps * HCH * w], F16, kind="ExternalOutput")
        )

    with tile.TileContext(nc) as tc, ExitStack() as ctx:
        const = ctx.enter_context(tc.tile_pool(name="const", bufs=1))
        psum = ctx.enter_context(tc.tile_pool(name="psum", bufs=1, space="PSUM"))

        w12_sb = const.tile([128, HCH * HCH * 128], F16)
        w1du_sb = const.tile([KD, HCH * 128], F16)
        w1_sb = const.tile([128, H], F16)
        xu0_sb = const.tile([128, BLOCAL], F16, name="xu0")
        du_sb = const.tile([KD, (t_steps - 1) * BLOCAL], F16, name="du")
        h_sb = []
        for s in range(streams):
            w = widths[s]
            h_sb.append(const.tile([128, RING * HCH * w], F16, name=f"h{s}"))

        psum_pre = []
        for s in range(streams):
            w = widths[s]
            psum_pre.append(psum.tile([128, HCH * w], F32, tag=f"pre{s}", name=f"pre{s}"))

        # Startup DMAs in dependency-deadline order.  w12 (8KB/partition,
        # the long pole gating burst 0) goes on the idle ACT queue so it
        # streams concurrently with the SP-queue seeds; du chunks after
        # the first alternate SP/Pool.
        nc.scalar.dma_start(w12_sb[:], d_w12.ap()[:])
        nc.sync.dma_start(xu0_sb[:], d_xu0.ap()[:])
        nc.sync.dma_start(w1_sb[:], d_w1.ap()[:])
        first = min(8, t_steps - 1)
        nc.sync.dma_start(
            du_sb[:, 0 : first * BLOCAL], d_du.ap()[:, 0 : first * BLOCAL]
        )
        nc.gpsimd.dma_start(w1du_sb[:], d_w1du.ap()[:])
        cb = np.linspace(first, t_steps - 1, 5).astype(int)
        for k in range(len(cb) - 1):
            a, b = int(cb[k]), int(cb[k + 1])
            if a == b:
                continue
            q = (nc.sync, nc.gpsimd)[k % 2]
            q.dma_start(
                du_sb[:, a * BLOCAL : b * BLOCAL], d_du.ap()[:, a * BLOCAL : b * BLOCAL]
            )

        # Warm-up: pull the ACT tanh table load off step 0's critical path
        # and give the PE p-state ramp something to chew on.
        warm_ps = psum.tile([SD, SD], F32, tag="warm", name="warm_ps")
        warm_sb = const.tile([SD, SD], F16, name="warm_sb")
        zero_sb = const.tile([128, 256], F16, name="zero_sb")
        nc.vector.memset(zero_sb[:], 0.0)
        for _ in range(4):
            nc.tensor.matmul(
                warm_ps[:], zero_sb[:, 0:SD], zero_sb[:, 0:SD], start=True, stop=True
            )
        nc.scalar.activation(warm_sb[:], warm_ps[:], mybir.ActivationFunctionType.Tanh)

        # Zero psum_pre with a single start=True matmul per tile, then seed
        # pre_0 = W1aug.T @ [x0; 1; u0] with accumulating matmuls.
        for s in range(streams):
            w = widths[s]
            nc.tensor.matmul(
                psum_pre[s][:], zero_sb[:, 0:128], zero_sb[:, 0 : HCH * w],
                start=True, stop=False, skip_group_check=True,
            )
        for s in range(streams):
            w = widths[s]
            off = OFFS[s]
            for j in range(HCH):
                nc.tensor.matmul(
                    psum_pre[s][:, ts(j, w)],
                    w1_sb[:, ts(j, 128)],
                    xu0_sb[:, off : off + w],
                    start=False, stop=False, skip_group_check=True,
                )

        for t in range(t_steps):
            last_t = t == t_steps - 1
            r = t % RING
            for s in range(streams):
                w = widths[s]
                hs = h_sb[s][:, (r * HCH) * w : (r * HCH + HCH) * w]
                nc.scalar.activation(
                    hs, psum_pre[s][:], mybir.ActivationFunctionType.Tanh
                )
                if not last_t:
                    # recurrence burst: pre += h @ W12 (16 mm) then the
                    # du/bias inject (4 mm); the next tanh waits on these.
                    for j in range(HCH):
                        for i in range(HCH):
                            nc.tensor.matmul(
                                psum_pre[s][:, ts(j, w)],
                                w12_sb[:, ts(j * HCH + i, 128)],
                                h_sb[s][:, (r * HCH + i) * w : (r * HCH + i + 1) * w],
                                start=False, stop=False, skip_group_check=True,
                            )
                    off = t * BLOCAL + OFFS[s]
                    for j in range(HCH):
                        nc.tensor.matmul(
                            psum_pre[s][:, ts(j, w)],
                            w1du_sb[:, ts(j, 128)],
                            du_sb[:, off : off + w],
                            start=False, stop=t == t_steps - 2,
                            skip_group_check=True,
                        )
            # Drain the ring in aligned half-chunks, one DMA queue per
            # stream (SP/DVE/Pool).  A half drained at slot 7 is rewritten
            # 9 rounds later, so the WAR on the ring never stalls a tanh.
            if r % (RING // 2) == RING // 2 - 1 or last_t:
                lo_r = r - (r % (RING // 2))
                lo_t = t - (r % (RING // 2))
                for s in range(streams):
                    w = widths[s]
                    q = (nc.sync, nc.gpsimd, nc.sync)[s % 3]
                    q.dma_start(
                        d_h[s].ap()[:, lo_t * HCH * w : (t + 1) * HCH * w],
                        h_sb[s][:, lo_r * HCH * w : (r + 1) * HCH * w],
                    )

    nc.compile()
    return nc


_CACHE: dict = {}


def _get_module():
    if "nc" not in _CACHE:
        _CACHE["nc"] = _build_module()
    return _CACHE["nc"]


def _prep_inputs(x0, controls, W1, b1, W2, b2):
    """Host-side prep: shard, transpose, augment, cast. Returns in_maps."""
    f16 = np.float16
    W1 = np.asarray(W1, np.float32)
    b1 = np.asarray(b1, np.float32)
    W2 = np.asarray(W2, np.float32)
    b2 = np.asarray(b2, np.float32)
    x0 = np.asarray(x0, np.float32)
    controls = np.asarray(controls, np.float32)
    W1x, W1u = W1[:SD], W1[SD:]

    # W12 = DT * W2 @ W1x, chunked for the PE: w12[p, j, i, c] = W12[128i+p, 128j+c]
    W12 = (DT * (W2 @ W1x)).astype(np.float32)
    w12 = W12.reshape(HCH, 128, HCH, 128).transpose(1, 2, 0, 3)
    w12 = np.ascontiguousarray(w12).reshape(128, HCH * HCH * 128).astype(f16)
    # du-inject weights: rows 0..31 = W1u, row 32 = DT * b2 @ W1x
    cvec = (DT * (b2 @ W1x)).astype(np.float32)
    w1du = np.concatenate([W1u, cvec[None, :]], axis=0).astype(f16)  # [33, 512]
    w1aug = np.concatenate(
        [W1x, b1[None, :], W1u, np.zeros((128 - KX, H), np.float32)], axis=0
    ).astype(f16)

    x0T = x0.T.astype(f16)                              # [SD, B]
    u0T = controls[:, 0, :].T.astype(f16)               # [CD, B]
    du = controls[:, 1:, :] - controls[:, :-1, :]       # [B, T-1, CD] f32
    duT = du.transpose(2, 1, 0).astype(f16)             # [CD, T-1, B]

    in_maps = []
    for c in range(NCORES):
        m = {"w12": w12, "w1du": w1du, "w1aug": w1aug}
        cols = slice(c * BLOCAL, (c + 1) * BLOCAL)
        xu0 = np.zeros((128, BLOCAL), f16)
        xu0[0:SD] = x0T[:, cols]
        xu0[SD] = 1.0
        xu0[SD + 1 : KX] = u0T[:, cols]
        m["xu0"] = xu0
        dstream = np.concatenate(
            [duT[:, :, cols], np.ones((1, T - 1, BLOCAL), f16)], axis=0
        )  # [33, T-1, 128]
        m["du"] = np.ascontiguousarray(dstream).reshape(KD, (T - 1) * BLOCAL)
        in_maps.append(m)
    return in_maps


def kernel(x0, controls, W1, b1, W2, b2):
    nc = _get_module()
    in_maps = _prep_inputs(x0, controls, W1, b1, W2, b2)
    res = run_bass_kernel_spmd(nc, in_maps, core_ids=list(range(NCORES)))

    x0 = np.asarray(x0, np.float32)
    W2f = np.asarray(W2, np.float32)
    b2f = np.asarray(b2, np.float32)
    states = np.empty((B, T + 1, SD), np.float32)
    states[:, 0, :] = x0
    for c in range(NCORES):
        for s in range(STREAMS):
            w = WIDTHS[s]
            lo = c * BLOCAL + OFFS[s]
            hraw = np.asarray(res.results[c][f"h{s}"], np.float16)
            # h[p, t, i, col] -> H[t, col, 128i+p]
            Hs = hraw.reshape(128, T, HCH, w).transpose(1, 3, 2, 0).reshape(T, w, H)
            v = Hs.astype(np.float32) @ (DT * W2f) + DT * b2f  # [T, w, SD]
            states[lo : lo + w, 1:] = x0[lo : lo + w, None, :] + np.cumsum(
                v, axis=0
            ).transpose(1, 0, 2)
    return states
